# revision 12
# baseline (speedup 1.0000x reference)
"""Layer-normalized BiLSTM on 8 trn2 NeuronCores (batch-parallel SPMD).

Per-core shard: 4 batch rows, both directions, no collectives. fp32
throughout: the 512-step LN-LSTM recurrence is chaotic, so fp32-class
arithmetic is required to stay at the fp32 ensemble spread (~1.6e-2 L2)
from the reference; bf16 lands at ~3.8e-2 and fails the 2e-2 gate.

vs the original baseline kernel (~23us/step simulated):
- persistent half-bank PSUM accumulators; LN stats chase each 512-wide
  half so bn_stats overlaps the recurrence matmuls
- bhh seed matmuls for step s+1 issue right after step s's PSUM readers
  (off the critical path; keeps the PE warm). Their width matches the
  recurrence matmuls -- accumulation groups break if it does not.
- gates split [i|f]/[o]/[g] with the g-tanh raced ahead of the sigmoids
- c-path tanh fused with the LN apply via Act scale/bias
- h^T computed directly as o^T * tanh(c)^T from two PE transposes into
  the matmul operand layout (no HY materialization + transpose + copy)
- runner drops the dead zero-output upload (outputs are fully written;
  lowering_input_output_aliases=() meant those buffers were never read)
~7us/step simulated vs ~23us/step for the baseline.
"""

import numpy as np

import concourse.bass as bass
import concourse.mybir as mybir
import concourse.tile as tile
from concourse.bass import IndirectOffsetOnAxis

F32 = mybir.dt.float32
BF16 = mybir.dt.bfloat16
NPBF16 = np.float32  # fp32 build: chaotic recurrence needs fp32-class arithmetic
AX = mybir.AluOpType
AF = mybir.ActivationFunctionType

B, L, D, V, TO = 32, 512, 256, 50000, 48
NCORES = 8
BL = B // NCORES          # batch rows per core
G4 = 4 * D                # 1024 gate width
EPS = 1e-5

# gate permutation: reference order (i, f, g, o) -> device order (i, f, o, g)
_PERM = np.concatenate([np.arange(0, D), np.arange(D, 2 * D),
                        np.arange(3 * D, 4 * D), np.arange(2 * D, 3 * D)])

# weight-pack layout (element offsets into the flat "wpack" input);
# emb first: the indirect gather's dynamic AP must have offset 0
_PACK_SIZES = [V * D, 2 * 2 * 128 * G4, 2 * 2 * 128 * G4, 2 * G4, 2 * G4,
               4 * 128 * TO, TO, 128 * 128, 128, 2 * 36]
(O_EMB, O_WIH, O_WHH, O_BIH, O_BHH, O_WOUT, O_BOUT, O_IDN, O_ONE, O_BSEL) = \
    np.cumsum([0] + _PACK_SIZES)[:-1].tolist()
WPACK_N = int(np.sum(_PACK_SIZES))


def split_sem_waits(nc, max_waits=1):
    """walrus in this container rejects >max_waits sem waits per instruction;
    hoist the excess onto NoOps that run just before on the same engine."""
    for f in nc.m.functions:
        for b in f.blocks:
            new_insts = []
            for ins in b.instructions:
                si = ins.sync_info
                if si is not None and si.on_wait and len(si.on_wait) > max_waits:
                    waits = list(si.on_wait)
                    for j, w in enumerate(waits[max_waits:]):
                        nop = mybir.InstNoOp(name=f"{ins.name}-wsplit{j}", ins=[], outs=[])
                        nop.engine = ins.engine
                        nop.sync_info = mybir.SyncInfo(on_wait=[w], on_update=[])
                        new_insts.append(nop)
                    ins.sync_info = mybir.SyncInfo(
                        on_wait=waits[:max_waits], on_update=list(si.on_update or []))
                new_insts.append(ins)
            b.instructions = new_insts


def _ap(t, offset, dims):
    return bass.AP(tensor=t.tensor if isinstance(t, bass.AP) else t,
                   offset=offset, ap=[list(d) for d in dims])


def build_nc(T=L, do_phase_a=True, split=True, debug=False):
    nc = bass.Bass("TRN2", target_bir_lowering=False)
    NT = BL * T // 128        # token tiles per core (phase A/C)

    xi = nc.dram_tensor("xi", [BL * T], mybir.dt.int32, kind="ExternalInput")
    # weights + embedding packed into one blob: each extra jit input costs
    # ~4ms of axon dispatch wall per execute
    wpack = nc.dram_tensor("wpack", [WPACK_N], F32, kind="ExternalInput")
    igd = nc.dram_tensor("igd", [2, BL, T, G4], F32,
                         kind="ExternalOutput" if debug else "Internal")
    out = nc.dram_tensor("out", [BL, T, TO], F32, kind="ExternalOutput")
    htbo = nc.dram_tensor("htbo", [128, 2, 8, T], F32,
                          kind="ExternalOutput") if debug else None

    with tile.TileContext(nc) as tc:
        with tc.tile_pool(name="const", bufs=1) as cpool, \
             tc.tile_pool(name="big", bufs=1) as bigpool, \
             tc.tile_pool(name="pa", bufs=3) as papool, \
             tc.tile_pool(name="pb", bufs=4) as pbpool, \
             tc.tile_pool(name="st", bufs=4) as stpool:

            # ---- constants / weights to SBUF ----
            WIH = cpool.tile([128, 2, 2, G4], F32)
            WHH = cpool.tile([128, 2, 2, G4], F32)
            WOUT = cpool.tile([128, 4, TO], F32)
            BIH = cpool.tile([1, 2, G4], F32)
            BHH = cpool.tile([2, G4], F32)
            BOUT = cpool.tile([1, TO], F32)
            IDN = cpool.tile([128, 128], F32)
            ONE = cpool.tile([1, 128], F32)
            BSEL = cpool.tile([2, 36], F32)
            for k in range(2):
                for d in range(2):
                    nc.sync.dma_start(
                        WIH[:, k, d, :],
                        _ap(wpack, O_WIH + (k * 2 + d) * 128 * G4, [[G4, 128], [1, G4]]))
                    nc.sync.dma_start(
                        WHH[:, k, d, :],
                        _ap(wpack, O_WHH + (k * 2 + d) * 128 * G4, [[G4, 128], [1, G4]]))
            for q in range(4):
                nc.sync.dma_start(
                    WOUT[:, q, :], _ap(wpack, O_WOUT + q * 128 * TO, [[TO, 128], [1, TO]]))
            nc.sync.dma_start(BIH[0:1, :, :],
                              _ap(wpack, O_BIH, [[2 * G4, 1], [G4, 2], [1, G4]]))
            nc.sync.dma_start(BHH[:, :], _ap(wpack, O_BHH, [[G4, 2], [1, G4]]))
            nc.sync.dma_start(BOUT[0:1, :], _ap(wpack, O_BOUT, [[TO, 1], [1, TO]]))
            nc.sync.dma_start(IDN[:, :], _ap(wpack, O_IDN, [[128, 128], [1, 128]]))
            nc.sync.dma_start(ONE[0:1, :], _ap(wpack, O_ONE, [[128, 1], [1, 128]]))
            nc.sync.dma_start(BSEL[:, :], _ap(wpack, O_BSEL, [[36, 2], [1, 36]]))
            EPSC = cpool.tile([128, 1], F32)
            nc.vector.memset(EPSC[:, :], EPS)

            # h^T history [128, hc, (d,b), t] bf16; col(s,d) below
            HTB = bigpool.tile([128, 2, 8, T], F32)
            nc.vector.memset(HTB[:, :, :, :], 0.0)
            # current-step h^T fast path for the recurrence matmuls
            HCUR = bigpool.tile([128, 2, 8], F32)
            nc.vector.memset(HCUR[:, :, :], 0.0)

            # ---- Phase A ----
            if do_phase_a:
              with tc.tile_pool(name="pa_ps", bufs=2, space="PSUM") as papsum:
                  XIDX = cpool.tile([128, NT], mybir.dt.int32)
                  nc.sync.dma_start(
                      XIDX[:, :],
                      _ap(xi, 0, [[1, 128], [128, NT]]))
                  for i in range(NT):
                      XS = papool.tile([128, D], F32, tag="xs")
                      nc.gpsimd.indirect_dma_start(
                          out=XS[:, :], out_offset=None,
                          in_=_ap(wpack, O_EMB, [[D, V], [1, D]]),
                          in_offset=IndirectOffsetOnAxis(ap=XIDX[:, i:i + 1], axis=0))
                      XT = papool.tile([128, 2, 128], F32, tag="xt")
                      for k in range(2):
                          TP = papsum.tile([128, 128], F32, tag="tp")
                          nc.tensor.transpose(TP[:, :], XS[:, k * 128:(k + 1) * 128], IDN[:, :])
                          nc.vector.tensor_copy(XT[:, k, :], TP[:, :])
                      bb, t0 = i // (T // 128), (i % (T // 128)) * 128
                      for d in range(2):
                          PSA = papsum.tile([128, G4], F32, tag="psa")
                          for nb in range(2):
                              nc.tensor.matmul(
                                  PSA[:, nb * 512:(nb + 1) * 512], ONE[0:1, :],
                                  BIH[0:1, d, nb * 512:(nb + 1) * 512],
                                  start=True, stop=False, skip_group_check=True)
                          for k in range(2):
                              for nb in range(2):
                                  nc.tensor.matmul(
                                      PSA[:, nb * 512:(nb + 1) * 512], XT[:, k, :],
                                      WIH[:, k, d, nb * 512:(nb + 1) * 512],
                                      start=False, stop=(k == 1), skip_group_check=True)
                          BN = stpool.tile([128, 2, 6], F32, tag="bn_a")
                          MV = stpool.tile([128, 2], F32, tag="mv_a")
                          SDV = stpool.tile([128, 4], F32, tag="sc_a")
                          for nb in range(2):
                              nc.vector.bn_stats(BN[:, nb, :], PSA[:, nb * 512:(nb + 1) * 512])
                          nc.vector.bn_aggr(MV[:, :], BN[:, :, :])
                          nc.scalar.activation(SDV[:, 0:1], MV[:, 1:2], AF.Sqrt, bias=EPSC[0:128, 0:1])
                          nc.vector.reciprocal(SDV[:, 1:2], SDV[:, 0:1])
                          nc.vector.scalar_tensor_tensor(
                              SDV[:, 2:3], MV[:, 0:1], -1.0, SDV[:, 1:2],
                              op0=AX.mult, op1=AX.mult)
                          IGA = papool.tile([128, G4], F32, tag="iga")
                          nc.scalar.activation(IGA[:, :], PSA[:, :], AF.Identity,
                                               bias=SDV[:, 2:3], scale=SDV[:, 1:2])
                          nc.sync.dma_start(
                              _ap(igd, ((d * BL + bb) * T + t0) * G4, [[G4, 128], [1, G4]]),
                              IGA[:, :])

            # ---- Phase B ----
            # rows: fwd batch rows at partitions 0..4, rev at 32..36
            GC = bigpool.tile([36, 512], F32)     # [g | cs]
            nc.vector.memset(GC[:, :], 0.0)
            RSTC = bigpool.tile([36, 1], F32)
            nc.vector.memset(RSTC[:, :], 1.0)

            def col(s, d):
                # time index in HTB for (step s, dir d)
                return s if d == 0 else (T - 1 - s)

            QW = 256          # matmul / stats chunk width

            pbps_ctx = tc.tile_pool(name="pb_ps", bufs=1, space="PSUM")
            pbpsum = pbps_ctx.__enter__()
            tpps_ctx = tc.tile_pool(name="tp_ps", bufs=2, space="PSUM")
            tppsum = tpps_ctx.__enter__()
            ig_slots = []
            for _ in range(4):
                t_ = pbpool.tile([36, G4], F32, tag="ig")
                nc.vector.memset(t_[:, :], 0.0)
                ig_slots.append(t_)

            # persistent PSUM halves: PH[0] = [i|f] gates, PH[1] = [o|g]
            PH = [pbpsum.tile([36, 512], F32, tag=f"p{h}", name=f"PH{h}") for h in range(2)]

            def bias_matmuls(first=False):
                # seed both PSUM halves with bhh via BSEL selector rows; the
                # group is closed by the recurrence matmuls (or here at s==0)
                for h in range(2):
                    nc.tensor.matmul(PH[h][:, :], BSEL[:, :],
                                     BHH[:, h * 512:(h + 1) * 512],
                                     start=True, stop=first, skip_group_check=True)

            bias_matmuls(first=True)

            for s in range(T):
                IG = ig_slots[s % 4]
                # fwd rows read t=s, rev rows read t=T-1-s
                nc.sync.dma_start(
                    _ap(IG, 0, [[G4, 4], [1, G4]]),
                    _ap(igd, (0 * BL * T + s) * G4,
                        [[T * G4, 4], [1, G4]]))
                nc.sync.dma_start(
                    _ap(IG, 32 * IG.ap[0][0], [[G4, 4], [1, G4]]),
                    _ap(igd, (BL * T + (T - 1 - s)) * G4,
                        [[T * G4, 4], [1, G4]]))

                # recurrence matmuls, half-major so stats can chase each half
                if s > 0:
                    for h in range(2):
                        for d in range(2):
                            for k in range(2):
                                lhsT = HCUR[:, k, 4 * d:4 * d + 4]
                                nc.tensor.matmul(
                                    _ap(PH[h], 32 * d * PH[h].ap[0][0],
                                        [[PH[h].ap[0][0], 4], [1, 512]]),
                                    lhsT, WHH[:, k, d, h * 512:(h + 1) * 512],
                                    start=False, stop=(d == 1 and k == 1),
                                    tile_position=(0, 32 * d), skip_group_check=True)
                # stats over 1024 per row, halves pipelined behind the matmuls
                BN = stpool.tile([36, 2, 6], F32, tag="bn_h")
                MV = stpool.tile([36, 2], F32, tag="mv_h")
                SD = stpool.tile([36, 4], F32, tag="sc_h")
                for h in range(2):
                    nc.vector.bn_stats(BN[:, h, :], PH[h][:, :])
                nc.vector.bn_aggr(MV[:, :], BN[:, :, :])
                nc.scalar.activation(SD[:, 0:1], MV[:, 1:2], AF.Sqrt, bias=EPSC[0:36, 0:1])
                nc.vector.reciprocal(SD[:, 1:2], SD[:, 0:1])
                nc.vector.scalar_tensor_tensor(
                    SD[:, 2:3], MV[:, 0:1], -1.0, SD[:, 1:2], op0=AX.mult, op1=AX.mult)
                # gates = ig + (z - mu)*istd; fp32 keeps the PE scarce, so the
                # ig add runs on the DVE (Pool cannot read PSUM)
                GNIF = pbpool.tile([36, 512], F32, tag="gnif")
                GNG = pbpool.tile([36, 256], F32, tag="gng")
                GNO = pbpool.tile([36, 256], F32, tag="gno")
                nc.vector.scalar_tensor_tensor(
                    GNIF[:, :], PH[0][:, :], SD[:, 1:2], IG[:, 0:512], op0=AX.mult, op1=AX.add)
                nc.vector.scalar_tensor_tensor(
                    GNG[:, :], PH[1][:, 256:512], SD[:, 1:2], IG[:, 768:1024], op0=AX.mult, op1=AX.add)
                nc.vector.scalar_tensor_tensor(
                    GNO[:, :], PH[1][:, 0:256], SD[:, 1:2], IG[:, 512:768], op0=AX.mult, op1=AX.add)
                A = pbpool.tile([36, 512], F32, tag="a")
                AO = pbpool.tile([36, 256], F32, tag="ao")
                nc.scalar.activation(A[:, :], GNIF[:, :], AF.Sigmoid,
                                     bias=SD[:, 2:3], scale=1.0)
                nc.scalar.activation(GC[:, 0:256], GNG[:, :], AF.Tanh,
                                     bias=SD[:, 2:3], scale=1.0)
                nc.scalar.activation(AO[:, :], GNO[:, :], AF.Sigmoid,
                                     bias=SD[:, 2:3], scale=1.0)
                # bias matmuls for step s+1 go after this step's PSUM readers
                if s + 1 < T:
                    bias_matmuls()
                # o^T early (off critical path): h^T = o^T * tanh(c)^T later
                AOT = tppsum.tile([128, 2, 36], F32, tag="aot")
                for k in range(2):
                    nc.tensor.transpose(AOT[:, k, :], AO[:, k * 128:(k + 1) * 128], IDN[0:36, 0:36])
                AOS = pbpool.tile([128, 2, 36], F32, tag="aos")
                nc.vector.tensor_copy(AOS[:, :, :], AOT[:, :, :])
                # c path: f*c can start before tanh(g) is out of the Act engine
                PR = pbpool.tile([36, 512], F32, tag="pr")
                nc.vector.tensor_tensor(PR[:, 256:512], A[:, 256:512], GC[:, 256:512], op=AX.mult)
                nc.vector.tensor_tensor(PR[:, 0:256], A[:, 0:256], GC[:, 0:256], op=AX.mult)
                CR = pbpool.tile([36, 256], F32, tag="cr")
                nc.vector.tensor_tensor(CR[:, :], PR[:, 0:256], PR[:, 256:512], op=AX.add)
                BNC = stpool.tile([36, 6], F32, tag="bn_c")
                MVC = stpool.tile([36, 2], F32, tag="mv_c")
                SDC = stpool.tile([36, 2], F32, tag="sc_c")
                nc.vector.bn_stats(BNC[:, :], CR[:, :])
                nc.vector.bn_aggr(MVC[:, :], BNC[:, :])
                nc.scalar.activation(SDC[:, 1:2], MVC[:, 1:2], AF.Sqrt, bias=EPSC[0:36, 0:1])
                nc.vector.reciprocal(RSTC[:, 0:1], SDC[:, 1:2])
                nc.vector.scalar_tensor_tensor(
                    SDC[:, 0:1], MVC[:, 0:1], -1.0, RSTC[:, 0:1], op0=AX.mult, op1=AX.mult)
                # normalized c state (off critical path) and fused tanh (Act)
                nc.vector.tensor_scalar(GC[:, 256:512], CR[:, :], MVC[:, 0:1], RSTC[:, 0:1],
                                        op0=AX.subtract, op1=AX.mult)
                TH = pbpool.tile([36, 256], F32, tag="th")
                nc.scalar.activation(TH[:, :], CR[:, :], AF.Tanh,
                                     bias=SDC[:, 0:1], scale=RSTC[:, 0:1])
                THT = tppsum.tile([128, 2, 36], F32, tag="tht")
                for k in range(2):
                    nc.tensor.transpose(THT[:, k, :], TH[:, k * 128:(k + 1) * 128], IDN[0:36, 0:36])
                # h^T = o^T * tanh(c)^T straight into the matmul operand layout
                sel = [[36, 2], [32, 2], [1, 4]]   # (k, d, b) <- cols {0-3, 32-35}
                nc.vector.tensor_tensor(
                    _ap(HCUR, 0, [[2 * 8, 128], [8, 2], [4, 2], [1, 4]]),
                    _ap(AOS, 0, [[AOS.ap[0][0], 128]] + sel),
                    _ap(THT, 0, [[THT.ap[0][0], 128]] + sel), op=AX.mult)
                # history for phase C (off critical path)
                nc.vector.tensor_copy(
                    _ap(HTB, s, [[2 * 8 * T, 128], [8 * T, 2], [(T - 1 - 2 * s) + 4 * T, 2], [T, 4]]),
                    _ap(HCUR, 0, [[2 * 8, 128], [8, 2], [4, 2], [1, 4]]))

            tpps_ctx.__exit__(None, None, None)
            pbps_ctx.__exit__(None, None, None)
            if debug:
                nc.sync.dma_start(htbo[:, :, :, :], HTB[:, :, :, :])
            # ---- Phase C ----
            pcps_ctx = tc.tile_pool(name="pc_ps", bufs=2, space="PSUM")
            pcpsum = pcps_ctx.__enter__()
            for i in range(NT):
                bb, t0 = i // (T // 128), (i % (T // 128)) * 128
                LG = pcpsum.tile([128, TO], F32, tag="lg")
                nc.tensor.matmul(LG[:, :], ONE[0:1, :], BOUT[0:1, :], start=True, stop=False, skip_group_check=True)
                for d in range(2):
                    for k in range(2):
                        nc.tensor.matmul(
                            LG[:, :], HTB[:, k, 4 * d + bb, t0:t0 + 128],
                            WOUT[:, 2 * d + k, :], start=False,
                            stop=(d == 1 and k == 1), skip_group_check=True)
                MX = stpool.tile([128, 4], F32, tag="mx")
                nc.vector.tensor_reduce(MX[:, 0:1], LG[:, :], mybir.AxisListType.X, AX.max)
                nc.vector.tensor_scalar_mul(MX[:, 1:2], MX[:, 0:1], -1.0)
                EX = papool.tile([128, TO], F32, tag="ex")
                nc.scalar.activation(EX[:, :], LG[:, :], AF.Exp,
                                     bias=MX[:, 1:2], scale=1.0, accum_out=MX[:, 2:3])
                nc.scalar.activation(MX[:, 3:4], MX[:, 2:3], AF.Ln)
                OT = papool.tile([128, TO], F32, tag="ot")
                nc.vector.tensor_scalar(OT[:, :], LG[:, :], MX[:, 0:1], MX[:, 3:4],
                                        op0=AX.subtract, op1=AX.subtract)
                nc.sync.dma_start(
                    _ap(out, ((bb * T) + t0) * TO, [[TO, 128], [1, TO]]),
                    OT[:, :])
            pcps_ctx.__exit__(None, None, None)

    if split:
        split_sem_waits(nc)
    return nc


def prep_weights(inputs):
    """host-side marshalling: permute gates, transpose, shard, cast bf16."""
    def pg(w):   # permute gate rows of [4D, ...] or [4D]
        return np.ascontiguousarray(w[_PERM])

    # wih layout [k, d, 128, G4]
    wih = np.empty((2, 2, 128, G4), np.float32)
    whh = np.empty((2, 2, 128, G4), np.float32)
    for d, sfx in enumerate(("e", "r")):
        wt = pg(inputs[f"Wih_{sfx}"]).T  # [256, 1024]
        ht = pg(inputs[f"Whh_{sfx}"]).T
        for k in range(2):
            wih[k, d] = wt[k * 128:(k + 1) * 128].astype(np.float32)
            whh[k, d] = ht[k * 128:(k + 1) * 128].astype(np.float32)
    bihs = np.stack([pg(inputs["bih_e"]), pg(inputs["bih_r"])]).astype(np.float32)
    bhhs = np.stack([pg(inputs["bhh_e"]), pg(inputs["bhh_r"])]).astype(np.float32)
    # wout [4=(d,hc), 128, TO]; h_cat = [xe | xr]
    wt = inputs["Wout"].T.astype(np.float32)  # [512, 48]
    woutp = np.stack([wt[(d * 2 + k) * 128:(d * 2 + k + 1) * 128]
                      for d in range(2) for k in range(2)]).astype(np.float32)
    bsel = np.zeros((2, 36), np.float32)
    bsel[0, 0:32] = 1.0
    bsel[1, 32:36] = 1.0
    wpack = np.concatenate([
        np.asarray(inputs["emb"]).astype(np.float32).ravel(),
        wih.ravel(), whh.ravel(), bihs.ravel(), bhhs.ravel(),
        woutp.ravel(), inputs["bout"].astype(np.float32).ravel(),
        np.eye(128, dtype=np.float32).ravel(),
        np.ones(128, np.float32), bsel.ravel(),
    ]).astype(np.float32)
    assert wpack.size == WPACK_N
    return {"wpack": wpack}


class _Runner:
    """compile once, execute many (run_bass_via_pjrt with a cached jit).

    Outputs are NOT donated/zero-seeded: this kernel writes every element
    of its single ExternalOutput, and with lowering_input_output_aliases=()
    the zero buffers were dead operands anyway. Dropping them removes a
    3.1MB host->device upload from every execute."""

    def __init__(self, nc):
        import jax
        from jax.sharding import Mesh, PartitionSpec
        from jax.experimental.shard_map import shard_map
        from concourse import bass2jax

        bass2jax.install_neuronx_cc_hook()
        self.jax = jax
        self._nc = nc
        partition_name = nc.partition_id_tensor.name if nc.partition_id_tensor else None
        in_names, out_names, out_avals = [], [], []
        import concourse.mybir as mb
        for alloc in nc.m.functions[0].allocations:
            if not isinstance(alloc, mb.MemoryLocationSet):
                continue
            name = alloc.memorylocations[0].name
            if alloc.kind == "ExternalInput":
                if name != partition_name:
                    in_names.append(name)
            elif alloc.kind == "ExternalOutput":
                out_names.append(name)
                shape = tuple(alloc.tensor_shape)
                dtype = mb.dt.np(alloc.dtype)
                out_avals.append(jax.core.ShapedArray(shape, dtype))
        self.in_names, self.out_names = in_names, out_names
        self._out_avals = out_avals
        all_in = list(in_names)
        if partition_name is not None:
            all_in = all_in + [partition_name]

        def _body(*args):
            operands = list(args)
            if partition_name is not None:
                operands.append(bass2jax.partition_id_tensor())
            outs = bass2jax._bass_exec_p.bind(
                *operands, out_avals=tuple(out_avals), in_names=tuple(all_in),
                out_names=tuple(out_names), lowering_input_output_aliases=(),
                sim_require_finite=False, sim_require_nnan=False, nc=nc)
            return tuple(outs)

        devices = jax.devices()[:NCORES]
        mesh = Mesh(np.asarray(devices), ("core",))
        in_specs = (PartitionSpec("core"),) * len(in_names)
        out_specs = (PartitionSpec("core"),) * len(out_names)
        self.fn = jax.jit(
            shard_map(_body, mesh=mesh, in_specs=in_specs, out_specs=out_specs,
                      check_rep=False),
            keep_unused=True)

    def build_rep(self, nrep):
        """jit fn executing the NEFF nrep times back-to-back in one dispatch;
        optimization_barrier threads a fake dependency so XLA neither CSEs
        nor reorders the repeated custom calls."""
        import jax
        from jax.sharding import Mesh, PartitionSpec
        from jax.experimental.shard_map import shard_map
        from concourse import bass2jax

        nc = self._nc
        partition_name = nc.partition_id_tensor.name if nc.partition_id_tensor else None
        all_in = list(self.in_names)
        if partition_name is not None:
            all_in = all_in + [partition_name]
        out_avals = self._out_avals

        def _body(*args):
            operands = list(args)
            if partition_name is not None:
                operands.append(bass2jax.partition_id_tensor())
            outs = None
            for _ in range(nrep):
                if outs is not None:
                    chained = jax.lax.optimization_barrier(
                        (operands[0], outs[0]))
                    operands = [chained[0]] + operands[1:]
                outs = bass2jax._bass_exec_p.bind(
                    *operands, out_avals=tuple(out_avals), in_names=tuple(all_in),
                    out_names=tuple(self.out_names),
                    lowering_input_output_aliases=(),
                    sim_require_finite=False, sim_require_nnan=False, nc=nc)
            return tuple(outs)

        devices = self.jax.devices()[:NCORES]
        mesh = Mesh(np.asarray(devices), ("core",))
        in_specs = (PartitionSpec("core"),) * len(self.in_names)
        out_specs = (PartitionSpec("core"),) * len(self.out_names)
        fn = jax.jit(
            shard_map(_body, mesh=mesh, in_specs=in_specs, out_specs=out_specs,
                      check_rep=False),
            keep_unused=True)
        return lambda: fn(*self.staged)

    def stage(self, in_maps):
        per_core = [[np.asarray(m[n]) for n in self.in_names] for m in in_maps]
        concat_in = [np.concatenate([per_core[c][i] for c in range(NCORES)], axis=0)
                     for i in range(len(self.in_names))]
        self.staged = [self.jax.device_put(a) for a in concat_in]
        for a in self.staged:
            a.block_until_ready()

    def execute_device(self):
        """dispatch + run on device; returns device arrays (no D2H)."""
        outs = self.fn(*self.staged)
        for o in outs:
            o.block_until_ready()
        return outs

    def fetch(self, outs):
        res = []
        for o in outs:
            a = np.asarray(o)
            res.append(np.split(a, NCORES, axis=0))
        return [{n: res[i][c] for i, n in enumerate(self.out_names)}
                for c in range(NCORES)]

    def run(self, in_maps):
        self.stage(in_maps)
        return self.fetch(self.execute_device())


_CACHE = {}


def _get_runner():
    if "r" not in _CACHE:
        _CACHE["r"] = _Runner(build_nc(L))
    return _CACHE["r"]


def _make_in_maps(inputs):
    shared = prep_weights(inputs)
    x = np.asarray(inputs["x"]).reshape(B, L).astype(np.int32)
    in_maps = []
    for c in range(NCORES):
        m = dict(shared)
        m["xi"] = np.ascontiguousarray(x[c * BL:(c + 1) * BL].reshape(-1))
        in_maps.append(m)
    return in_maps


def kernel(**inputs):
    r = _get_runner()
    in_maps = _make_in_maps(inputs)
    res = r.run(in_maps)
    return np.concatenate([res[c]["out"] for c in range(NCORES)], axis=0)


def kernel_rerun():
    """re-execute with inputs already staged on device (timing helper).
    Returns device arrays; D2H is the caller's concern."""
    return _CACHE["r"].execute_device()


def kernel_fetch(outs):
    r = _CACHE["r"]
    res = r.fetch(outs)
    return np.concatenate([res[c]["out"] for c in range(NCORES)], axis=0)


# revision 13
# speedup vs baseline: 1.4525x; 1.4525x over previous
"""Layer-normalized BiLSTM on 8 trn2 NeuronCores (batch-parallel SPMD).

Per-core shard: 4 batch rows, both directions, no collectives. fp32
throughout: the 512-step LN-LSTM recurrence is chaotic, so fp32-class
arithmetic is required to stay at the fp32 ensemble spread (~1.6e-2 L2)
from the reference; bf16 lands at ~3.8e-2 and fails the 2e-2 gate.

vs the original baseline kernel (~23us/step simulated):
- persistent half-bank PSUM accumulators; LN stats chase each 512-wide
  half so bn_stats overlaps the recurrence matmuls
- bhh seed matmuls for step s+1 issue right after step s's PSUM readers
  (off the critical path; keeps the PE warm). Their width matches the
  recurrence matmuls -- accumulation groups break if it does not.
- gates split [i|f]/[o]/[g] with the g-tanh raced ahead of the sigmoids
- c-path tanh fused with the LN apply via Act scale/bias
- h^T computed directly as o^T * tanh(c)^T from two PE transposes into
  the matmul operand layout (no HY materialization + transpose + copy)
- runner drops the dead zero-output upload (outputs are fully written;
  lowering_input_output_aliases=() meant those buffers were never read)
~7us/step simulated vs ~23us/step for the baseline.
"""

import numpy as np

import concourse.bass as bass
import concourse.mybir as mybir
import concourse.tile as tile
from concourse.bass import IndirectOffsetOnAxis

F32 = mybir.dt.float32
BF16 = mybir.dt.bfloat16
NPBF16 = np.float32  # fp32 build: chaotic recurrence needs fp32-class arithmetic
AX = mybir.AluOpType
AF = mybir.ActivationFunctionType

B, L, D, V, TO = 32, 512, 256, 50000, 48
NCORES = 8
BL = B // NCORES          # batch rows per core
G4 = 4 * D                # 1024 gate width
EPS = 1e-5

# gate permutation: reference order (i, f, g, o) -> device order (i, f, o, g)
_PERM = np.concatenate([np.arange(0, D), np.arange(D, 2 * D),
                        np.arange(3 * D, 4 * D), np.arange(2 * D, 3 * D)])

# weight-pack layout (element offsets into the flat "wpack" input)
_PACK_SIZES = [2 * 2 * 128 * G4, 2 * 2 * 128 * G4, 2 * G4, 2 * G4,
               4 * 128 * TO, TO, 128 * 128, 128, 2 * 36]
(O_WIH, O_WHH, O_BIH, O_BHH, O_WOUT, O_BOUT, O_IDN, O_ONE, O_BSEL) = \
    np.cumsum([0] + _PACK_SIZES)[:-1].tolist()
WPACK_N = int(np.sum(_PACK_SIZES))


def split_sem_waits(nc, max_waits=1):
    """walrus in this container rejects >max_waits sem waits per instruction;
    hoist the excess onto NoOps that run just before on the same engine."""
    for f in nc.m.functions:
        for b in f.blocks:
            new_insts = []
            for ins in b.instructions:
                si = ins.sync_info
                if si is not None and si.on_wait and len(si.on_wait) > max_waits:
                    waits = list(si.on_wait)
                    for j, w in enumerate(waits[max_waits:]):
                        nop = mybir.InstNoOp(name=f"{ins.name}-wsplit{j}", ins=[], outs=[])
                        nop.engine = ins.engine
                        nop.sync_info = mybir.SyncInfo(on_wait=[w], on_update=[])
                        new_insts.append(nop)
                    ins.sync_info = mybir.SyncInfo(
                        on_wait=waits[:max_waits], on_update=list(si.on_update or []))
                new_insts.append(ins)
            b.instructions = new_insts


def _ap(t, offset, dims):
    return bass.AP(tensor=t.tensor if isinstance(t, bass.AP) else t,
                   offset=offset, ap=[list(d) for d in dims])


def build_nc(T=L, do_phase_a=True, split=True, debug=False):
    nc = bass.Bass("TRN2", target_bir_lowering=False)
    NT = BL * T // 128        # token tiles per core (phase A/C)

    emb = nc.dram_tensor("emb", [V, D], F32, kind="ExternalInput")
    xi = nc.dram_tensor("xi", [BL * T], mybir.dt.int32, kind="ExternalInput")
    # small weights packed into one blob: each extra jit input costs
    # ~4ms of axon dispatch wall per execute
    wpack = nc.dram_tensor("wpack", [WPACK_N], F32, kind="ExternalInput")
    igd = nc.dram_tensor("igd", [2, BL, T, G4], F32,
                         kind="ExternalOutput" if debug else "Internal")
    out = nc.dram_tensor("out", [BL, T, TO], F32, kind="ExternalOutput")
    htbo = nc.dram_tensor("htbo", [128, 2, 8, T], F32,
                          kind="ExternalOutput") if debug else None

    with tile.TileContext(nc) as tc:
        with tc.tile_pool(name="const", bufs=1) as cpool, \
             tc.tile_pool(name="big", bufs=1) as bigpool, \
             tc.tile_pool(name="pa", bufs=3) as papool, \
             tc.tile_pool(name="pb", bufs=4) as pbpool, \
             tc.tile_pool(name="st", bufs=4) as stpool:

            # ---- constants / weights to SBUF ----
            WIH = cpool.tile([128, 2, 2, G4], F32)
            WHH = cpool.tile([128, 2, 2, G4], F32)
            WOUT = cpool.tile([128, 4, TO], F32)
            BIH = cpool.tile([1, 2, G4], F32)
            BHH = cpool.tile([2, G4], F32)
            BOUT = cpool.tile([1, TO], F32)
            IDN = cpool.tile([128, 128], F32)
            ONE = cpool.tile([1, 128], F32)
            BSEL = cpool.tile([2, 36], F32)
            for k in range(2):
                for d in range(2):
                    nc.sync.dma_start(
                        WIH[:, k, d, :],
                        _ap(wpack, O_WIH + (k * 2 + d) * 128 * G4, [[G4, 128], [1, G4]]))
                    nc.sync.dma_start(
                        WHH[:, k, d, :],
                        _ap(wpack, O_WHH + (k * 2 + d) * 128 * G4, [[G4, 128], [1, G4]]))
            for q in range(4):
                nc.sync.dma_start(
                    WOUT[:, q, :], _ap(wpack, O_WOUT + q * 128 * TO, [[TO, 128], [1, TO]]))
            nc.sync.dma_start(BIH[0:1, :, :],
                              _ap(wpack, O_BIH, [[2 * G4, 1], [G4, 2], [1, G4]]))
            nc.sync.dma_start(BHH[:, :], _ap(wpack, O_BHH, [[G4, 2], [1, G4]]))
            nc.sync.dma_start(BOUT[0:1, :], _ap(wpack, O_BOUT, [[TO, 1], [1, TO]]))
            nc.sync.dma_start(IDN[:, :], _ap(wpack, O_IDN, [[128, 128], [1, 128]]))
            nc.sync.dma_start(ONE[0:1, :], _ap(wpack, O_ONE, [[128, 1], [1, 128]]))
            nc.sync.dma_start(BSEL[:, :], _ap(wpack, O_BSEL, [[36, 2], [1, 36]]))
            EPSC = cpool.tile([128, 1], F32)
            nc.vector.memset(EPSC[:, :], EPS)

            # h^T history [128, hc, (d,b), t] bf16; col(s,d) below
            HTB = bigpool.tile([128, 2, 8, T], F32)
            nc.vector.memset(HTB[:, :, :, :], 0.0)
            # current-step h^T fast path for the recurrence matmuls
            HCUR = bigpool.tile([128, 2, 8], F32)
            nc.vector.memset(HCUR[:, :, :], 0.0)

            # ---- Phase A ----
            if do_phase_a:
              with tc.tile_pool(name="pa_ps", bufs=2, space="PSUM") as papsum:
                  XIDX = cpool.tile([128, NT], mybir.dt.int32)
                  nc.sync.dma_start(
                      XIDX[:, :],
                      _ap(xi, 0, [[1, 128], [128, NT]]))
                  for i in range(NT):
                      XS = papool.tile([128, D], F32, tag="xs")
                      nc.gpsimd.indirect_dma_start(
                          out=XS[:, :], out_offset=None, in_=emb[:, :],
                          in_offset=IndirectOffsetOnAxis(ap=XIDX[:, i:i + 1], axis=0))
                      XT = papool.tile([128, 2, 128], F32, tag="xt")
                      for k in range(2):
                          TP = papsum.tile([128, 128], F32, tag="tp")
                          nc.tensor.transpose(TP[:, :], XS[:, k * 128:(k + 1) * 128], IDN[:, :])
                          nc.vector.tensor_copy(XT[:, k, :], TP[:, :])
                      bb, t0 = i // (T // 128), (i % (T // 128)) * 128
                      for d in range(2):
                          PSA = papsum.tile([128, G4], F32, tag="psa")
                          for nb in range(2):
                              nc.tensor.matmul(
                                  PSA[:, nb * 512:(nb + 1) * 512], ONE[0:1, :],
                                  BIH[0:1, d, nb * 512:(nb + 1) * 512],
                                  start=True, stop=False, skip_group_check=True)
                          for k in range(2):
                              for nb in range(2):
                                  nc.tensor.matmul(
                                      PSA[:, nb * 512:(nb + 1) * 512], XT[:, k, :],
                                      WIH[:, k, d, nb * 512:(nb + 1) * 512],
                                      start=False, stop=(k == 1), skip_group_check=True)
                          BN = stpool.tile([128, 2, 6], F32, tag="bn_a")
                          MV = stpool.tile([128, 2], F32, tag="mv_a")
                          SDV = stpool.tile([128, 4], F32, tag="sc_a")
                          for nb in range(2):
                              nc.vector.bn_stats(BN[:, nb, :], PSA[:, nb * 512:(nb + 1) * 512])
                          nc.vector.bn_aggr(MV[:, :], BN[:, :, :])
                          nc.scalar.activation(SDV[:, 0:1], MV[:, 1:2], AF.Sqrt, bias=EPSC[0:128, 0:1])
                          nc.vector.reciprocal(SDV[:, 1:2], SDV[:, 0:1])
                          nc.vector.scalar_tensor_tensor(
                              SDV[:, 2:3], MV[:, 0:1], -1.0, SDV[:, 1:2],
                              op0=AX.mult, op1=AX.mult)
                          IGA = papool.tile([128, G4], F32, tag="iga")
                          nc.scalar.activation(IGA[:, :], PSA[:, :], AF.Identity,
                                               bias=SDV[:, 2:3], scale=SDV[:, 1:2])
                          nc.sync.dma_start(
                              _ap(igd, ((d * BL + bb) * T + t0) * G4, [[G4, 128], [1, G4]]),
                              IGA[:, :])

            # ---- Phase B ----
            # rows: fwd batch rows at partitions 0..4, rev at 32..36
            GC = bigpool.tile([36, 512], F32)     # [g | cs]
            nc.vector.memset(GC[:, :], 0.0)
            RSTC = bigpool.tile([36, 1], F32)
            nc.vector.memset(RSTC[:, :], 1.0)

            def col(s, d):
                # time index in HTB for (step s, dir d)
                return s if d == 0 else (T - 1 - s)

            QW = 256          # matmul / stats chunk width

            pbps_ctx = tc.tile_pool(name="pb_ps", bufs=1, space="PSUM")
            pbpsum = pbps_ctx.__enter__()
            tpps_ctx = tc.tile_pool(name="tp_ps", bufs=2, space="PSUM")
            tppsum = tpps_ctx.__enter__()
            ig_slots = []
            for _ in range(4):
                t_ = pbpool.tile([36, G4], F32, tag="ig")
                nc.vector.memset(t_[:, :], 0.0)
                ig_slots.append(t_)

            # persistent PSUM halves: PH[0] = [i|f] gates, PH[1] = [o|g]
            PH = [pbpsum.tile([36, 512], F32, tag=f"p{h}", name=f"PH{h}") for h in range(2)]

            def bias_matmuls(first=False):
                # seed both PSUM halves with bhh via BSEL selector rows; the
                # group is closed by the recurrence matmuls (or here at s==0)
                for h in range(2):
                    nc.tensor.matmul(PH[h][:, :], BSEL[:, :],
                                     BHH[:, h * 512:(h + 1) * 512],
                                     start=True, stop=first, skip_group_check=True)

            bias_matmuls(first=True)

            for s in range(T):
                IG = ig_slots[s % 4]
                # fwd rows read t=s, rev rows read t=T-1-s
                nc.sync.dma_start(
                    _ap(IG, 0, [[G4, 4], [1, G4]]),
                    _ap(igd, (0 * BL * T + s) * G4,
                        [[T * G4, 4], [1, G4]]))
                nc.sync.dma_start(
                    _ap(IG, 32 * IG.ap[0][0], [[G4, 4], [1, G4]]),
                    _ap(igd, (BL * T + (T - 1 - s)) * G4,
                        [[T * G4, 4], [1, G4]]))

                # recurrence matmuls, half-major so stats can chase each half
                if s > 0:
                    for h in range(2):
                        for d in range(2):
                            for k in range(2):
                                lhsT = HCUR[:, k, 4 * d:4 * d + 4]
                                nc.tensor.matmul(
                                    _ap(PH[h], 32 * d * PH[h].ap[0][0],
                                        [[PH[h].ap[0][0], 4], [1, 512]]),
                                    lhsT, WHH[:, k, d, h * 512:(h + 1) * 512],
                                    start=False, stop=(d == 1 and k == 1),
                                    tile_position=(0, 32 * d), skip_group_check=True)
                # stats over 1024 per row, halves pipelined behind the matmuls
                BN = stpool.tile([36, 2, 6], F32, tag="bn_h")
                MV = stpool.tile([36, 2], F32, tag="mv_h")
                SD = stpool.tile([36, 4], F32, tag="sc_h")
                for h in range(2):
                    nc.vector.bn_stats(BN[:, h, :], PH[h][:, :])
                nc.vector.bn_aggr(MV[:, :], BN[:, :, :])
                nc.scalar.activation(SD[:, 0:1], MV[:, 1:2], AF.Sqrt, bias=EPSC[0:36, 0:1])
                nc.vector.reciprocal(SD[:, 1:2], SD[:, 0:1])
                nc.vector.scalar_tensor_tensor(
                    SD[:, 2:3], MV[:, 0:1], -1.0, SD[:, 1:2], op0=AX.mult, op1=AX.mult)
                # gates = ig + (z - mu)*istd; fp32 keeps the PE scarce, so the
                # ig add runs on the DVE (Pool cannot read PSUM)
                GNIF = pbpool.tile([36, 512], F32, tag="gnif")
                GNG = pbpool.tile([36, 256], F32, tag="gng")
                GNO = pbpool.tile([36, 256], F32, tag="gno")
                nc.vector.scalar_tensor_tensor(
                    GNIF[:, :], PH[0][:, :], SD[:, 1:2], IG[:, 0:512], op0=AX.mult, op1=AX.add)
                nc.vector.scalar_tensor_tensor(
                    GNG[:, :], PH[1][:, 256:512], SD[:, 1:2], IG[:, 768:1024], op0=AX.mult, op1=AX.add)
                nc.vector.scalar_tensor_tensor(
                    GNO[:, :], PH[1][:, 0:256], SD[:, 1:2], IG[:, 512:768], op0=AX.mult, op1=AX.add)
                A = pbpool.tile([36, 512], F32, tag="a")
                AO = pbpool.tile([36, 256], F32, tag="ao")
                nc.scalar.activation(A[:, :], GNIF[:, :], AF.Sigmoid,
                                     bias=SD[:, 2:3], scale=1.0)
                nc.scalar.activation(GC[:, 0:256], GNG[:, :], AF.Tanh,
                                     bias=SD[:, 2:3], scale=1.0)
                nc.scalar.activation(AO[:, :], GNO[:, :], AF.Sigmoid,
                                     bias=SD[:, 2:3], scale=1.0)
                # bias matmuls for step s+1 go after this step's PSUM readers
                if s + 1 < T:
                    bias_matmuls()
                # o^T early (off critical path): h^T = o^T * tanh(c)^T later
                AOT = tppsum.tile([128, 2, 36], F32, tag="aot")
                for k in range(2):
                    nc.tensor.transpose(AOT[:, k, :], AO[:, k * 128:(k + 1) * 128], IDN[0:36, 0:36])
                AOS = pbpool.tile([128, 2, 36], F32, tag="aos")
                nc.vector.tensor_copy(AOS[:, :, :], AOT[:, :, :])
                # c path: f*c can start before tanh(g) is out of the Act engine
                PR = pbpool.tile([36, 512], F32, tag="pr")
                nc.vector.tensor_tensor(PR[:, 256:512], A[:, 256:512], GC[:, 256:512], op=AX.mult)
                nc.vector.tensor_tensor(PR[:, 0:256], A[:, 0:256], GC[:, 0:256], op=AX.mult)
                CR = pbpool.tile([36, 256], F32, tag="cr")
                nc.vector.tensor_tensor(CR[:, :], PR[:, 0:256], PR[:, 256:512], op=AX.add)
                BNC = stpool.tile([36, 6], F32, tag="bn_c")
                MVC = stpool.tile([36, 2], F32, tag="mv_c")
                SDC = stpool.tile([36, 2], F32, tag="sc_c")
                nc.vector.bn_stats(BNC[:, :], CR[:, :])
                nc.vector.bn_aggr(MVC[:, :], BNC[:, :])
                nc.scalar.activation(SDC[:, 1:2], MVC[:, 1:2], AF.Sqrt, bias=EPSC[0:36, 0:1])
                nc.vector.reciprocal(RSTC[:, 0:1], SDC[:, 1:2])
                nc.vector.scalar_tensor_tensor(
                    SDC[:, 0:1], MVC[:, 0:1], -1.0, RSTC[:, 0:1], op0=AX.mult, op1=AX.mult)
                # normalized c state (off critical path) and fused tanh (Act)
                nc.vector.tensor_scalar(GC[:, 256:512], CR[:, :], MVC[:, 0:1], RSTC[:, 0:1],
                                        op0=AX.subtract, op1=AX.mult)
                TH = pbpool.tile([36, 256], F32, tag="th")
                nc.scalar.activation(TH[:, :], CR[:, :], AF.Tanh,
                                     bias=SDC[:, 0:1], scale=RSTC[:, 0:1])
                THT = tppsum.tile([128, 2, 36], F32, tag="tht")
                for k in range(2):
                    nc.tensor.transpose(THT[:, k, :], TH[:, k * 128:(k + 1) * 128], IDN[0:36, 0:36])
                # h^T = o^T * tanh(c)^T straight into the matmul operand layout
                sel = [[36, 2], [32, 2], [1, 4]]   # (k, d, b) <- cols {0-3, 32-35}
                nc.vector.tensor_tensor(
                    _ap(HCUR, 0, [[2 * 8, 128], [8, 2], [4, 2], [1, 4]]),
                    _ap(AOS, 0, [[AOS.ap[0][0], 128]] + sel),
                    _ap(THT, 0, [[THT.ap[0][0], 128]] + sel), op=AX.mult)
                # history for phase C (off critical path)
                nc.vector.tensor_copy(
                    _ap(HTB, s, [[2 * 8 * T, 128], [8 * T, 2], [(T - 1 - 2 * s) + 4 * T, 2], [T, 4]]),
                    _ap(HCUR, 0, [[2 * 8, 128], [8, 2], [4, 2], [1, 4]]))

            tpps_ctx.__exit__(None, None, None)
            pbps_ctx.__exit__(None, None, None)
            if debug:
                nc.sync.dma_start(htbo[:, :, :, :], HTB[:, :, :, :])
            # ---- Phase C ----
            pcps_ctx = tc.tile_pool(name="pc_ps", bufs=2, space="PSUM")
            pcpsum = pcps_ctx.__enter__()
            for i in range(NT):
                bb, t0 = i // (T // 128), (i % (T // 128)) * 128
                LG = pcpsum.tile([128, TO], F32, tag="lg")
                nc.tensor.matmul(LG[:, :], ONE[0:1, :], BOUT[0:1, :], start=True, stop=False, skip_group_check=True)
                for d in range(2):
                    for k in range(2):
                        nc.tensor.matmul(
                            LG[:, :], HTB[:, k, 4 * d + bb, t0:t0 + 128],
                            WOUT[:, 2 * d + k, :], start=False,
                            stop=(d == 1 and k == 1), skip_group_check=True)
                MX = stpool.tile([128, 4], F32, tag="mx")
                nc.vector.tensor_reduce(MX[:, 0:1], LG[:, :], mybir.AxisListType.X, AX.max)
                nc.vector.tensor_scalar_mul(MX[:, 1:2], MX[:, 0:1], -1.0)
                EX = papool.tile([128, TO], F32, tag="ex")
                nc.scalar.activation(EX[:, :], LG[:, :], AF.Exp,
                                     bias=MX[:, 1:2], scale=1.0, accum_out=MX[:, 2:3])
                nc.scalar.activation(MX[:, 3:4], MX[:, 2:3], AF.Ln)
                OT = papool.tile([128, TO], F32, tag="ot")
                nc.vector.tensor_scalar(OT[:, :], LG[:, :], MX[:, 0:1], MX[:, 3:4],
                                        op0=AX.subtract, op1=AX.subtract)
                nc.sync.dma_start(
                    _ap(out, ((bb * T) + t0) * TO, [[TO, 128], [1, TO]]),
                    OT[:, :])
            pcps_ctx.__exit__(None, None, None)

    if split:
        split_sem_waits(nc)
    return nc


def prep_weights(inputs):
    """host-side marshalling: permute gates, transpose, shard, cast bf16."""
    def pg(w):   # permute gate rows of [4D, ...] or [4D]
        return np.ascontiguousarray(w[_PERM])

    # wih layout [k, d, 128, G4]
    wih = np.empty((2, 2, 128, G4), np.float32)
    whh = np.empty((2, 2, 128, G4), np.float32)
    for d, sfx in enumerate(("e", "r")):
        wt = pg(inputs[f"Wih_{sfx}"]).T  # [256, 1024]
        ht = pg(inputs[f"Whh_{sfx}"]).T
        for k in range(2):
            wih[k, d] = wt[k * 128:(k + 1) * 128].astype(np.float32)
            whh[k, d] = ht[k * 128:(k + 1) * 128].astype(np.float32)
    bihs = np.stack([pg(inputs["bih_e"]), pg(inputs["bih_r"])]).astype(np.float32)
    bhhs = np.stack([pg(inputs["bhh_e"]), pg(inputs["bhh_r"])]).astype(np.float32)
    # wout [4=(d,hc), 128, TO]; h_cat = [xe | xr]
    wt = inputs["Wout"].T.astype(np.float32)  # [512, 48]
    woutp = np.stack([wt[(d * 2 + k) * 128:(d * 2 + k + 1) * 128]
                      for d in range(2) for k in range(2)]).astype(np.float32)
    bsel = np.zeros((2, 36), np.float32)
    bsel[0, 0:32] = 1.0
    bsel[1, 32:36] = 1.0
    wpack = np.concatenate([
        wih.ravel(), whh.ravel(), bihs.ravel(), bhhs.ravel(),
        woutp.ravel(), inputs["bout"].astype(np.float32).ravel(),
        np.eye(128, dtype=np.float32).ravel(),
        np.ones(128, np.float32), bsel.ravel(),
    ]).astype(np.float32)
    assert wpack.size == WPACK_N
    return {
        "emb": np.asarray(inputs["emb"]).astype(np.float32),
        "wpack": wpack,
    }


class _Runner:
    """compile once, execute many (run_bass_via_pjrt with a cached jit).

    Outputs are NOT donated/zero-seeded: this kernel writes every element
    of its single ExternalOutput, and with lowering_input_output_aliases=()
    the zero buffers were dead operands anyway. Dropping them removes a
    3.1MB host->device upload from every execute."""

    def __init__(self, nc):
        import jax
        from jax.sharding import Mesh, PartitionSpec
        from jax.experimental.shard_map import shard_map
        from concourse import bass2jax

        bass2jax.install_neuronx_cc_hook()
        self.jax = jax
        self._nc = nc
        partition_name = nc.partition_id_tensor.name if nc.partition_id_tensor else None
        in_names, out_names, out_avals = [], [], []
        import concourse.mybir as mb
        for alloc in nc.m.functions[0].allocations:
            if not isinstance(alloc, mb.MemoryLocationSet):
                continue
            name = alloc.memorylocations[0].name
            if alloc.kind == "ExternalInput":
                if name != partition_name:
                    in_names.append(name)
            elif alloc.kind == "ExternalOutput":
                out_names.append(name)
                shape = tuple(alloc.tensor_shape)
                dtype = mb.dt.np(alloc.dtype)
                out_avals.append(jax.core.ShapedArray(shape, dtype))
        self.in_names, self.out_names = in_names, out_names
        self._out_avals = out_avals
        all_in = list(in_names)
        if partition_name is not None:
            all_in = all_in + [partition_name]

        def _body(*args):
            operands = list(args)
            if partition_name is not None:
                operands.append(bass2jax.partition_id_tensor())
            outs = bass2jax._bass_exec_p.bind(
                *operands, out_avals=tuple(out_avals), in_names=tuple(all_in),
                out_names=tuple(out_names), lowering_input_output_aliases=(),
                sim_require_finite=False, sim_require_nnan=False, nc=nc)
            return tuple(outs)

        devices = jax.devices()[:NCORES]
        mesh = Mesh(np.asarray(devices), ("core",))
        in_specs = (PartitionSpec("core"),) * len(in_names)
        out_specs = (PartitionSpec("core"),) * len(out_names)
        self.fn = jax.jit(
            shard_map(_body, mesh=mesh, in_specs=in_specs, out_specs=out_specs,
                      check_rep=False),
            keep_unused=True)

    def build_rep(self, nrep):
        """jit fn executing the NEFF nrep times back-to-back in one dispatch;
        optimization_barrier threads a fake dependency so XLA neither CSEs
        nor reorders the repeated custom calls."""
        import jax
        from jax.sharding import Mesh, PartitionSpec
        from jax.experimental.shard_map import shard_map
        from concourse import bass2jax

        nc = self._nc
        partition_name = nc.partition_id_tensor.name if nc.partition_id_tensor else None
        all_in = list(self.in_names)
        if partition_name is not None:
            all_in = all_in + [partition_name]
        out_avals = self._out_avals

        def _body(*args):
            operands = list(args)
            if partition_name is not None:
                operands.append(bass2jax.partition_id_tensor())
            outs = None
            for _ in range(nrep):
                if outs is not None:
                    chained = jax.lax.optimization_barrier(
                        (operands[0], outs[0]))
                    operands = [chained[0]] + operands[1:]
                outs = bass2jax._bass_exec_p.bind(
                    *operands, out_avals=tuple(out_avals), in_names=tuple(all_in),
                    out_names=tuple(self.out_names),
                    lowering_input_output_aliases=(),
                    sim_require_finite=False, sim_require_nnan=False, nc=nc)
            return tuple(outs)

        devices = self.jax.devices()[:NCORES]
        mesh = Mesh(np.asarray(devices), ("core",))
        in_specs = (PartitionSpec("core"),) * len(self.in_names)
        out_specs = (PartitionSpec("core"),) * len(self.out_names)
        fn = jax.jit(
            shard_map(_body, mesh=mesh, in_specs=in_specs, out_specs=out_specs,
                      check_rep=False),
            keep_unused=True)
        return lambda: fn(*self.staged)

    def stage(self, in_maps):
        per_core = [[np.asarray(m[n]) for n in self.in_names] for m in in_maps]
        concat_in = [np.concatenate([per_core[c][i] for c in range(NCORES)], axis=0)
                     for i in range(len(self.in_names))]
        self.staged = [self.jax.device_put(a) for a in concat_in]
        for a in self.staged:
            a.block_until_ready()

    def execute_device(self):
        """dispatch + run on device; returns device arrays (no D2H)."""
        outs = self.fn(*self.staged)
        for o in outs:
            o.block_until_ready()
        return outs

    def fetch(self, outs):
        res = []
        for o in outs:
            a = np.asarray(o)
            res.append(np.split(a, NCORES, axis=0))
        return [{n: res[i][c] for i, n in enumerate(self.out_names)}
                for c in range(NCORES)]

    def run(self, in_maps):
        self.stage(in_maps)
        return self.fetch(self.execute_device())


_CACHE = {}


def _get_runner():
    if "r" not in _CACHE:
        _CACHE["r"] = _Runner(build_nc(L))
    return _CACHE["r"]


def _make_in_maps(inputs):
    shared = prep_weights(inputs)
    x = np.asarray(inputs["x"]).reshape(B, L).astype(np.int32)
    in_maps = []
    for c in range(NCORES):
        m = dict(shared)
        m["xi"] = np.ascontiguousarray(x[c * BL:(c + 1) * BL].reshape(-1))
        in_maps.append(m)
    return in_maps


def kernel(**inputs):
    r = _get_runner()
    in_maps = _make_in_maps(inputs)
    res = r.run(in_maps)
    return np.concatenate([res[c]["out"] for c in range(NCORES)], axis=0)


def kernel_rerun():
    """re-execute with inputs already staged on device (timing helper).
    Returns device arrays; D2H is the caller's concern."""
    return _CACHE["r"].execute_device()


def kernel_fetch(outs):
    r = _CACHE["r"]
    res = r.fetch(outs)
    return np.concatenate([res[c]["out"] for c in range(NCORES)], axis=0)
